# revision 71
# baseline (speedup 1.0000x reference)
"""Luong 'concat' attention TRN2 Bass kernel.

Problem: B=64, S=2048, D=512 (enc_dim == dec_dim), fp32.
  hidden = tanh(enc @ W_enc^T + ht @ W_dec^T + W_b)   [B, S, D]
  scores = hidden @ V_w^T (+ V_b)                     [B, 1, S]
  weights = softmax(scores, axis=-1)
  c_t = weights @ enc                                 [B, 1, D]

Sharding: data-parallel over batch, 8 batches per core on 8 cores.

Per-core dataflow (enc read from HBM exactly once, fp32):
  1. DMA enc[b] natural-layout into SBUF (quarters of 512 rows); the 2MB
     weight load rides the Activation HWDGE queue so the SP queue can
     start streaming enc immediately.
  2. PE transposes (f32r, 1.5 cyc/row) -> PSUM; DVE copy-backs convert
     to fp8e4 (encT, e on partitions), split into two half-tiles so mm1
     can chain on the first half before the second lands.
  3. mm1 in fp8e4 DoubleRow perf mode (0.5 cyc/row, K=256 per matmul):
     hiddenT[d,s] accumulated over 2 k-pair chains, et2/et3 pair first
     (it is transposed first).  W_enc pre-scaled by 16 before fp8
     quantization; the 1/16 plus the decoder bias (ht@W_dec^T + W_b)[d,b]
     fold into the ACT tanh (scale= + bias=).
  4. scoresT via N=1 matmuls: the big hT tile is the *stationary*
     operand and the V column the moving one, so each matmul costs ~1
     row instead of 512: psT[s,1] accumulated over 4 d-chunks.
  5. exp on ACT (accum_out -> per-partition partial denominators; V_b
     dropped, softmax shift-invariant).  Weights stay *unnormalized*;
     denominators are folded into the output in the tail.
  6. c_tT via N=2 matmuls (fp32r needs an even moving count; the weight
     column is stride-0 broadcast and the twin column discarded), enc
     natural tiles stationary, exp-weights moving: 64 short matmuls per
     batch instead of streaming 16x512 rows.
  7. Tail: one ones-matmul totals all 8 denominators, broadcast-expand
     x4, transpose, reciprocal, scale the transposed c_tT, one DMA out.

Windows of 1024 s-positions (2/batch) let each tanh cover [128,1024]
(two PSUM banks) amortizing ACT fixed latency.  The batch loop is
software-pipelined: transposes of window k overlap mm1 of window k-1,
scores lag two windows, and exp/c_t of batch b run in window 2b+4.
PSUM: 3 banks transpose staging + 2x2 mm1/tanh + 1 scores.
"""

import numpy as np

B, S, D = 64, 2048, 512
N_CORES = 8
BPC = B // N_CORES       # batches per core
ET = D // 128            # 4 e-chunks
DT = D // 128            # 4 d-chunks
NW = 2                   # s-windows per batch (1024 wide)
WIN = S // NW            # 1024
NQ = 4                   # enc DMA quarters per batch (512 rows each)
SB = S // 128            # 16 s-blocks of 128 per batch

_CACHE = {}


def _build(mm1_fp8=True):
    import concourse.bacc as bacc
    import concourse.tile as tile
    from concourse import mybir
    import concourse.bass as bass
    from concourse.masks import make_identity

    f32 = mybir.dt.float32
    f32r = mybir.dt.float32r
    bf16 = mybir.dt.bfloat16
    fp8 = mybir.dt.float8e4
    Tanh = mybir.ActivationFunctionType.Tanh
    Exp = mybir.ActivationFunctionType.Exp
    Copy = mybir.ActivationFunctionType.Copy
    DR = mybir.MatmulPerfMode.DoubleRow

    W_SCALE = 16.0  # pre-scale W_enc before fp8 quantization

    nc = bacc.Bacc(None, target_bir_lowering=False, debug=False)
    enc = nc.dram_tensor("enc_outs", [BPC, S, D], f32, kind="ExternalInput").ap()
    ht = nc.dram_tensor("ht", [1, BPC, D], f32, kind="ExternalInput").ap()
    W_w = nc.dram_tensor("W_w", [D, 2 * D], f32, kind="ExternalInput").ap()
    W_b = nc.dram_tensor("W_b", [D], f32, kind="ExternalInput").ap()
    V_w = nc.dram_tensor("V_w", [1, D], f32, kind="ExternalInput").ap()
    out = nc.dram_tensor("c_t", [BPC, 1, D], f32, kind="ExternalOutput").ap()

    with tile.TileContext(nc) as tc:
        with tc.tile_pool(name="const", bufs=1) as const, \
             tc.tile_pool(name="enc_nat", bufs=16) as enc_pool, \
             tc.tile_pool(name="e8", bufs=2) as e8_pool, \
             tc.tile_pool(name="hT", bufs=8) as hT_pool, \
             tc.tile_pool(name="wx", bufs=2) as wx_pool, \
             tc.tile_pool(name="pp_t", bufs=3, space="PSUM") as pp_t, \
             tc.tile_pool(name="pp_h", bufs=2, space="PSUM") as pp_h, \
             tc.tile_pool(name="pp_s", bufs=1, space="PSUM") as pp_s:

            ident = const.tile([128, 128], f32)
            make_identity(nc, ident)
            ident_r = const.tile([128, 128], f32r)
            nc.scalar.copy(out=ident_r, in_=ident)
            ones = const.tile([128, 128], f32)
            nc.gpsimd.memset(ones, 1.0)

            # ---- weights: load natural, transpose on PE ----
            # W_enc half -> fp8 (x16) for DoubleRow mm1; W_dec half -> f32r
            # for the decoder-bias matmul.  The 2MB W load leads the queue:
            # each dma_start costs ~565ns of SP sequencer time, so the tiny
            # rows would otherwise leave the DMA device idle at the start.
            wnat = [const.tile([128, DT, D], f32r, name=f"wnat{h}")
                    for h in range(2)]
            w8 = const.tile([128, ET, D], fp8, name="w8")
            w_decT = [const.tile([128, D], f32r, tag=f"w_decT{i}", name=f"w_decT{i}")
                      for i in range(ET)]
            for half in range(2):
                nc.scalar.dma_start(
                    out=wnat[half],
                    in_=W_w[:, half * D:(half + 1) * D].rearrange(
                        "(dc p) e -> p dc e", p=128).bitcast(f32r),
                )

            # ---- small input rows ----
            vrow = const.tile([1, D], f32)
            nc.sync.dma_start(out=vrow, in_=V_w)
            wbrow = const.tile([1, D], f32)
            nc.sync.dma_start(out=wbrow, in_=W_b.rearrange("(o d) -> o d", o=1))
            htn = const.tile([BPC, D], f32)
            nc.sync.dma_start(out=htn, in_=ht[0])

            def emit_W_setup(half):
                for ec in range(ET):
                    pt = pp_t.tile([128, D], f32r, tag="pt", name=f"ptw{half}_{ec}")
                    for dc in range(DT):
                        nc.tensor.transpose(
                            out=pt[:, dc * 128:(dc + 1) * 128],
                            in_=wnat[half][:, dc, ec * 128:(ec + 1) * 128],
                            identity=ident_r,
                        )
                    if half == 0:
                        nc.scalar.activation(out=w8[:, ec, :], in_=pt,
                                             func=Copy, scale=W_SCALE)
                    else:
                        nc.scalar.copy(out=w_decT[ec], in_=pt)

            emit_W_setup(0)
            emit_W_setup(1)

            # ---- V as per-partition columns [128, DT], bf16 ----
            v_pcol = const.tile([128, DT], f32)
            for dt_i in range(DT):
                pv = pp_s.tile([128, 16], f32, tag="bt", name=f"pv{dt_i}")
                nc.tensor.transpose(
                    out=pv[:, 0:1],
                    in_=vrow[0:1, dt_i * 128:(dt_i + 1) * 128],
                    identity=ident[0:1, 0:1],
                )
                nc.scalar.copy(out=v_pcol[:, dt_i:dt_i + 1], in_=pv[:, 0:1])
            v_bf = const.tile([128, DT], bf16)
            nc.vector.tensor_copy(out=v_bf, in_=v_pcol)

            # ---- W_b as per-partition columns ----
            wb_pcol = const.tile([128, DT], f32)
            for dc in range(DT):
                pv = pp_s.tile([128, 16], f32, tag="bt", name=f"pvb{dc}")
                nc.tensor.transpose(
                    out=pv[:, 0:1],
                    in_=wbrow[0:1, dc * 128:(dc + 1) * 128],
                    identity=ident[0:1, 0:1],
                )
                nc.scalar.copy(out=wb_pcol[:, dc:dc + 1], in_=pv[:, 0:1])

            # ---- decoder bias  bias_db[dc][d, b] = (ht@W_dec^T + W_b)[d, b] ----
            htT = const.tile([128, ET, BPC], f32r)
            for ec in range(ET):
                pv = pp_s.tile([128, 16], f32, tag="bt", name=f"pvh{ec}")
                nc.tensor.transpose(
                    out=pv[:, 0:BPC],
                    in_=htn[:, ec * 128:(ec + 1) * 128],
                    identity=ident[0:BPC, 0:BPC],
                )
                nc.scalar.copy(out=htT[:, ec, :], in_=pv[:, 0:BPC])
            bias_db = [const.tile([128, BPC], f32, tag=f"bias{i}", name=f"bias{i}")
                       for i in range(DT)]
            for dc in range(DT):
                pb = pp_s.tile([128, 16], f32, tag="bt", name=f"pbias{dc}")
                for ec in range(ET):
                    nc.tensor.matmul(
                        out=pb[:, 0:BPC],
                        lhsT=w_decT[ec][:, dc * 128:(dc + 1) * 128],
                        rhs=htT[:, ec, :],
                        start=(ec == 0), stop=(ec == ET - 1),
                    )
                nc.vector.tensor_scalar_add(
                    out=bias_db[dc], in0=pb[:, 0:BPC], scalar1=wb_pcol[:, dc:dc + 1]
                )

            # ---- per-core accumulators for the tail ----
            ct_all = const.tile([128, BPC * ET], f32r)   # c_tT, col = b*ET+ec
            rden_all = const.tile([128, BPC], f32)       # 1/denominator per b

            # ================= software-pipelined main loop =================
            # window k = 0..15: batch k//2, half k%2.  Per iteration k:
            #   exp/denominator of batch (k-4)//2 (k even), then c_t chains
            #   DMA for batch k//2+2 (k even)
            #   transposes T(k) interleaved with mm1+tanh M(k-1)
            #   scoresT S(k-2)
            eq = {}       # (b, q) -> enc natural quarter tile
            e8t = {}      # k -> fp8 encT window tile
            hT = {}       # (k, dt) -> tanh output tile
            bt = {}       # b -> small psum tile: [0:16] psT, [16:17] den, [20:24] ctT
            wexp = {}     # b -> unnormalized softmax weights [128, SB] f32r

            def emit_dma(b):
                for q in range(NQ):
                    t = enc_pool.tile([128, NQ, D], f32r, tag="eq",
                                      name=f"eq{b}_{q}")
                    eq[(b, q)] = t
                    nc.sync.dma_start(
                        out=t,
                        in_=enc[b, q * D:(q + 1) * D, :].rearrange(
                            "(sb p) e -> p sb e", p=128).bitcast(f32r),
                    )

            def emit_T_unit(k, et, half):
                # transpose 4 [128,128] blocks of window k into psum, copy to
                # fp8 encT.  half selects 512 of the 1024-wide window.
                b, w = k // 2, k % 2
                pt = pp_t.tile([128, 512], f32r, tag="pt", name=f"pt{k}_{et}_{half}")
                for j in range(4):
                    sb = w * 8 + half * 4 + j          # global s-block in batch
                    nc.tensor.transpose(
                        out=pt[:, j * 128:(j + 1) * 128],
                        in_=eq[(b, sb // 4)][:, sb % 4, et * 128:(et + 1) * 128],
                        identity=ident_r,
                    )
                grp = e8t[(k, et // 2, half)]
                dst = grp[:, et % 2, :]
                nc.vector.tensor_copy(out=dst, in_=pt)

            def emit_M_unit(k, dt_i):
                # mm1 (DoubleRow fp8) + fused bias/tanh for window k, d-chunk
                # dt_i.  The ep=1 half (et2/et3, transposed first) leads so the
                # chain's first dependency is ready mid-window.
                b = k // 2
                ph = pp_h.tile([128, 2, 512], f32, tag="ph", name=f"ph{k}_{dt_i}")
                for half in range(2):
                    for ep in (1, 0):
                        nc.tensor.matmul(
                            out=ph[:, half, :],
                            lhsT=w8[:, 2 * ep:2 * ep + 2, dt_i * 128:(dt_i + 1) * 128],
                            rhs=e8t[(k, ep, half)][:, 0:2, :],
                            perf_mode=DR,
                            start=(ep == 1), stop=(ep == 0),
                        )
                h = hT_pool.tile([128, 2, 512], bf16, tag="hT", name=f"hT{k}_{dt_i}")
                nc.scalar.activation(
                    out=h.rearrange("p a b -> p (a b)"),
                    in_=ph.rearrange("p a b -> p (a b)"),
                    func=Tanh, scale=1.0 / W_SCALE, bias=bias_db[dt_i][:, b:b + 1],
                )
                hT[(k, dt_i)] = h

            def emit_S(k):
                # scoresT for window k: 8 s-chunks of 128, each a 4-deep
                # N=1 accumulation with hT stationary and V moving.
                b, w = k // 2, k % 2
                if w == 0:
                    bt[b] = pp_s.tile([128, 16], f32, tag="bt", name=f"bt{b}")
                for scj in range(8):
                    col = w * 8 + scj
                    for dt_i in range(DT):
                        nc.tensor.matmul(
                            out=bt[b][:, col:col + 1],
                            lhsT=hT[(k, dt_i)][:, scj // 4,
                                               (scj % 4) * 128:(scj % 4) * 128 + 128],
                            rhs=v_bf[:, dt_i:dt_i + 1],
                            start=(dt_i == 0), stop=(dt_i == DT - 1),
                        )

            def emit_X_exp(b):
                # exp -> unnormalized weights + per-partition sums for batch b
                wx = wx_pool.tile([128, SB], f32r, tag="wx", name=f"wx{b}")
                wexp[b] = wx
                with tc.high_priority():
                    nc.scalar.activation(
                        out=wx, in_=bt[b][:, 0:SB], func=Exp,
                        accum_out=den1s[:, b:b + 1],
                    )

            def emit_CT(b):
                # c_tT[e, b*4+ec] = sum_s wexp[s] * enc[s, e], enc stationary.
                # fp32r matmuls need an even moving/dest inner count, so the
                # weight column is broadcast to N=2 (second column discarded).
                pd = pp_t.tile([128, 2 * ET + 2], f32, tag="pt", name=f"pd{b}")
                for ec in range(ET):
                    for sb in range(SB):
                        src = wexp[b][:, sb:sb + 1]
                        src2 = bass.AP(tensor=src.tensor, offset=src.offset,
                                       ap=[src.ap[0], [0, 2]])
                        nc.tensor.matmul(
                            out=pd[:, 2 * ec:2 * ec + 2],
                            lhsT=eq[(b, sb // 4)][:, sb % 4, ec * 128:(ec + 1) * 128],
                            rhs=src2,
                            start=(sb == 0), stop=(sb == SB - 1),
                        )
                # denominator: the ones-matmul broadcasts the batch total to
                # every partition, so the softmax division folds into the
                # psum->SBUF copy as a per-partition scalar
                dsrc = den1s[:, b:b + 1]
                dsrc2 = bass.AP(tensor=dsrc.tensor, offset=dsrc.offset,
                                ap=[dsrc.ap[0], [0, 2]])
                nc.tensor.matmul(out=pd[:, 2 * ET:2 * ET + 2], lhsT=ones,
                                 rhs=dsrc2, start=True, stop=True)
                nc.vector.reciprocal(out=rden_all[:, b:b + 1],
                                     in_=pd[:, 2 * ET:2 * ET + 1])

                ctq = pd.rearrange("p (e two) -> p e two", two=2)
                nc.vector.tensor_scalar_mul(
                    out=ct_all[:, b * ET:(b + 1) * ET],
                    in0=ctq[:, 0:ET, 0], scalar1=rden_all[:, b:b + 1])

            den1s = const.tile([128, BPC], f32)   # per-partition exp sums

            emit_dma(0)
            emit_dma(1)
            NK = NW * BPC
            for k in range(NK + 2):
                if k % 2 == 0 and k // 2 + 2 < BPC:
                    emit_dma(k // 2 + 2)
                # interleave: 8 T units with 4 M units (2:1)
                mq = list(range(DT)) if 1 <= k <= NK else []
                if k < NK:
                    # fp8 encT as four [et-pair, half] tiles so each mm1 chain
                    # depends only on the quarter it actually reads
                    for gp in range(2):
                        for gh in range(2):
                            e8t[(k, gp, gh)] = e8_pool.tile(
                                [128, 2, WIN // 2], fp8, tag=f"e8_{gp}{gh}",
                                name=f"e8_{k}_{gp}{gh}")
                    order = [(3, 0), (3, 1), (2, 0), (2, 1),
                             (0, 0), (0, 1), (1, 0), (1, 1)]
                    for u, (et, half) in enumerate(order):
                        emit_T_unit(k, et, half)
                        if u % 2 == 1 and mq:
                            emit_M_unit(k - 1, mq.pop(0))
                while mq:
                    emit_M_unit(k - 1, mq.pop(0))
                # bt[b]'s only reader is the exp right below, so the single
                # PSUM bank is free for S's next-batch tile here
                if 2 <= k < NK + 2:
                    emit_S(k - 2)
                    if (k - 2) % 2 == 1:
                        # batch (k-3)//2 fully scored: softmax + c_t now
                        emit_X_exp((k - 3) // 2)
                        emit_CT((k - 3) // 2)

            # ---- tail: transpose the (already normalized) c_tT, DMA out ----
            pf = pp_t.tile([128, 512], f32r, tag="pt", name="pf")
            nc.tensor.transpose(
                out=pf[0:BPC * ET, 0:128],
                in_=ct_all,
                identity=ident_r,
            )
            ct_out = const.tile([BPC * ET, 128], f32)
            nc.vector.tensor_copy(out=ct_out,
                                  in_=pf[0:BPC * ET, 0:128].bitcast(f32))
            nc.sync.dma_start(
                out=out.rearrange("b o (ec p) -> (b o ec) p", p=128),
                in_=ct_out,
            )

    nc.compile()
    return nc


def _get_nc():
    if "nc" not in _CACHE:
        _CACHE["nc"] = _build()
    return _CACHE["nc"]


def _run(inputs, trace=False, **kw):
    from concourse.bass_utils import run_bass_kernel_spmd

    nc = _get_nc()
    enc = np.asarray(inputs["enc_outs"], dtype=np.float32)
    ht = np.asarray(inputs["ht"], dtype=np.float32)
    W_w = np.asarray(inputs["W_w"], dtype=np.float32)
    W_b = np.asarray(inputs["W_b"], dtype=np.float32)
    V_w = np.asarray(inputs["V_w"], dtype=np.float32)
    in_maps = []
    for c in range(N_CORES):
        sl = slice(c * BPC, (c + 1) * BPC)
        in_maps.append({
            "enc_outs": enc[sl],
            "ht": ht[:, sl],
            "W_w": W_w,
            "W_b": W_b,
            "V_w": V_w,
        })
    res = run_bass_kernel_spmd(
        nc, in_maps, core_ids=list(range(N_CORES)), trace=trace, **kw
    )
    full = np.concatenate([res.results[c]["c_t"] for c in range(N_CORES)], axis=0)
    return full, res


def kernel(**inputs) -> np.ndarray:
    out, _ = _run(inputs, trace=False)
    return out
